# revision 70
# baseline (speedup 1.0000x reference)
"""Trainium2 Bass kernel for nn_Attention_90074054132266 — v10.

Full multi-head attention (B=2, S=4096, D=512, H=8, HD=64) with RoPE on
q/k, sharded over 8 NeuronCores: batch x head-pair (core c: batch c//4,
heads 2*(c%4), 2*(c%4)+1). Host sums the 4 per-batch partial outputs and
adds wo_b.

v10 design (TimelineSim 236.4us vs v9's 312.6us / 340us HW; rel err
1.484e-02 HW-validated vs the 2e-2 gate):
  - exp split across ACT and DVE. ACT (the v9 bottleneck at ~266us
    busy: 256 x [128,1024] tiles x ~1.04us) keeps 11/16 tiles per
    (qt,h); DVE takes kc2 in {1,4,7,10,13} via the int16 Schraudolph
    bit-trick: int16 = s'' + (16256 + delta), bitcast bf16 ~= exp(s_raw/8),
    one tensor_scalar_add per tile. The 128*log2e/8 scale is folded into
    the host-scaled wq, so no clamp or multiply is needed (int16 range
    covers all scores; sim truncates / HW rounds = half-LSB, 0.27%).
    delta=-12 calibrated end-to-end on device.
  - PV computed TRANSPOSED: O[q,d] = P^T V as 65-row matmuls
    (stationary = exp'd score subtile [128k, 128q], moving = V' [128k,65])
    -> 8320 PE cycles per (qt,h) vs 16384. The ones-column of V' lands
    the softmax denominator Z on PARTITIONS: reciprocal reads PSUM
    directly and 1/Z folds into the evacuation (one stride-0-broadcast
    tensor_tensor), killing v9's Z-transpose/outer-product machinery.
    PV windows run j4-OUTER so the 4 accumulation groups in the pso bank
    stay sequential (a matmul start re-marks the whole 2KB zero region).
  - normalized O is PE-transposed back per head (identity permutation
    matmul -> bf16 PSUM, head h on partitions h*64..) and the output
    projection is ONE matmul per 128-q tile (both heads contract in one
    pass via the stacked-partition wo layout); f32 store via DMA.
  - PSUM rebudget: scores get THREE [128,1024] slots (6 banks) so the
    slower DVE-exp tiles never stall the PE score stream; the p1 ring
    drops to 1 bank (rope/transpose/out-proj chains serialize in filler
    slack) and both heads share one pso bank sequentially.
  - projection phase: all K chunks first (attention exp stream
    unthrottles ~3x earlier), V projections deferred at filler priority
    into the PSS ring; v-evac on the (idle) ACT; k/q rope via
    permutation matmuls (5 PE matmuls per chunk instead of 8) with
    rope adds and zero-pad memsets on the idle GPSIMD (Pool).
Engine busy per core: PE 202us, ACT 195us, DVE 155us, Pool ~35us.
"""

import os
import sys

sys.path.insert(0, "/opt/trn_rl_repo")

import numpy as np

B, S, DIM, HEADS, HD = 2, 4096, 512, 8, 64
HALF = HD // 2
NCORES = 8
HPC = 2  # heads per core
DPC = HPC * HD  # 128 projection columns per core
NSC = S // 512  # 8 q-column chunks of 512
NKC = S // 128  # 32 k-chunks of 128
NUT = S // 128  # 32 q-row tiles of 128
KC2 = NKC // 2  # 16 pairs of k-chunks (exp batches of [128, 1024])
VW = HPC * (HD + 1)  # 130: per-k V' row for both heads (64+1 each)

LOG2E = 1.4426950408889634
QSCALE = 128.0 * LOG2E / 8.0  # folded into wq/wqp host-side
ACT_SCALE = 1.0 / (8.0 * QSCALE)  # exp(s_raw/8) = exp(s'' * ACT_SCALE)
B16 = 16256.0 + float(os.environ.get("ATTN_B16_DELTA", "-12"))  # 127*128 + delta
# which kc2 tiles go to the DVE Schraudolph path (rest go to ACT exp)
DVE_KC2 = tuple(
    int(t)
    for t in os.environ.get("ATTN_DVE_KC2", "1,4,7,10,13").split(",")
    if t != ""
)

_CACHE = {}


def _split_multiwait_drains(nc):
    """The walrus build in this container rejects any instruction with
    more than one sync-wait ("Too many sync wait commands"). Hoist the
    extra waits onto preceding same-engine NoOps, leaving one wait on
    the original instruction."""
    import bass_rust
    import concourse.mybir as mybir

    for fn in nc.m.functions:
        for bb in fn.blocks:
            new_insts = []
            changed = False
            for inst in bb.instructions:
                si = getattr(inst, "sync_info", None)
                if si is not None and len(si.on_wait) > 1:
                    waits = list(si.on_wait)
                    for k, w in enumerate(waits[:-1]):
                        d = mybir.InstNoOp(name=f"{inst.name}w{k}", ins=[], outs=[])
                        d.engine = inst.engine
                        d.sync_info = bass_rust.SyncInfo(on_wait=[w], on_update=[])
                        new_insts.append(d)
                    inst.sync_info = bass_rust.SyncInfo(
                        on_wait=[waits[-1]], on_update=list(si.on_update)
                    )
                    changed = True
                new_insts.append(inst)
            if changed:
                bb.instructions = new_insts


def _build(qk_bias, v_bias, use_bf16=True):
    import concourse.bass as bass
    import concourse.tile as tile
    from concourse import mybir

    F32 = mybir.dt.float32
    BF16 = mybir.dt.bfloat16
    I16 = mybir.dt.int16
    MMD = BF16
    MME = BF16
    EXP = mybir.ActivationFunctionType.Exp
    MUL = mybir.AluOpType.mult
    ADD = mybir.AluOpType.add

    nc = bass.Bass("TRN2")
    use_perm = not qk_bias

    xt_e = nc.declare_dram_parameter("xt", [DIM, S], MME, isOutput=False)
    w_e = {}
    w_names = ("wq", "wqp", "wk", "wv") if use_perm else ("wq", "wqp", "wk", "wkp", "wv")
    for name in w_names:
        w_e[name] = nc.declare_dram_parameter(name, [DIM, DPC], MME, isOutput=False)
    if use_perm:
        perm_e = nc.declare_dram_parameter("permm", [DPC, DPC], MME, isOutput=False)
    ident_e = nc.declare_dram_parameter("ident", [128, 128], MME, isOutput=False)
    wo_e = nc.declare_dram_parameter("wo", [DPC, DIM], MME, isOutput=False)
    # compact rope tables: cos rows replicate 4x (HALF=32 unique rows),
    # sin rows 2x ([-sin32; sin32]); expanded to 128 partitions by the
    # DMA broadcast access pattern.
    cos_e = nc.declare_dram_parameter("cosf", [HALF, S], MME, isOutput=False)
    sin_e = nc.declare_dram_parameter("sinf", [2 * HALF, S], MME, isOutput=False)
    b_e = {}
    if qk_bias:
        for name in ("qb", "qbp", "kb", "kbp"):
            b_e[name] = nc.declare_dram_parameter(name, [DPC, 1], F32, isOutput=False)
    if v_bias:
        b_e["vb"] = nc.declare_dram_parameter("vb", [1, DPC], F32, isOutput=False)
    out_e = nc.declare_dram_parameter("out", [S, DIM], F32, isOutput=True)

    with tile.TileContext(nc) as tc:
        with (
            tc.tile_pool(name="persist", bufs=1) as P,
            tc.tile_pool(name="work", bufs=4) as W,
        ):
            # ---- persistent SBUF tensors ----
            qr = P.tile([DPC, S], MMD, tag="qr")  # rotated q^T (x QSCALE)
            # rotated k^T, zero-padded per head to full K=128 contraction
            krA = P.tile([DPC, S], MMD, tag="krA")
            krB = P.tile([DPC, S], MMD, tag="krB")
            nc.gpsimd.memset(krA[HD:DPC, :], 0.0)
            nc.gpsimd.memset(krB[0:HD, :], 0.0)
            # V' rows: per k-chunk st, V[k, :] for head A cols 0:64 + ones
            # col 64, head B cols 65:129 + ones col 129.
            vb_sb = P.tile([128, NKC, VW], MMD, tag="vboth")

            bias_sb = {}
            if qk_bias:
                for name in ("qb", "qbp", "kb", "kbp"):
                    t = P.tile([DPC, 1], F32, tag=name)
                    nc.sync.dma_start(out=t, in_=b_e[name][:])
                    bias_sb[name] = t
            if v_bias:
                vbias_bc = P.tile([128, DPC], F32, tag="vbias")
                src = bass.AP(
                    tensor=b_e["vb"].tensor,
                    offset=b_e["vb"].offset,
                    ap=[[0, 128], [1, DPC]],
                )
                nc.sync.dma_start(out=vbias_bc, in_=src)

            # ---- PSUM budget (8 banks): p1 ring 2 + scores 4 + o/u 2
            with (
                tc.tile_pool(name="xtp", bufs=10) as XT,
                tc.tile_pool(name="wpool", bufs=1) as WP,
                tc.tile_pool(name="pps", bufs=1, space="PSUM") as PPS,
                tc.tile_pool(name="pss", bufs=3, space="PSUM") as PSS,
                tc.tile_pool(name="pou", bufs=1, space="PSUM") as POU,
            ):
                w_sb = {}

                def load_w(name):
                    t = WP.tile([128, 4, DPC], MMD, tag=name)
                    nc.sync.dma_start(
                        out=t,
                        in_=w_e[name][:].rearrange("(c p) m -> p c m", p=128),
                    )
                    w_sb[name] = t

                with tc.high_priority():
                    load_w("wq")
                    load_w("wqp")
                cos_sb = WP.tile([DPC, S], MME, tag="cos")
                sin_sb = WP.tile([DPC, S], MME, tag="sin")

                def load_trig(cs, eng=None):
                    (eng or nc.sync).dma_start(
                        out=cos_sb[:, cs],
                        in_=bass.AP(
                            tensor=cos_e[:].tensor,
                            offset=cos_e[:].offset + cs.start,
                            ap=[[0, 4], [S, HALF], [1, 512]],
                        ),
                    )
                    (eng or nc.sync).dma_start(
                        out=sin_sb[:, cs],
                        in_=bass.AP(
                            tensor=sin_e[:].tensor,
                            offset=sin_e[:].offset + cs.start,
                            ap=[[0, 2], [S, 2 * HALF], [1, 512]],
                        ),
                    )

                # ones columns of V' (written once; disjoint from evac cols)
                ones_ap = vb_sb[:].rearrange("p s (j w) -> p s j w", w=HD + 1)[
                    :, :, :, HD : HD + 1
                ]
                nc.vector.memset(ones_ap, 1.0)

                xt_r = xt_e[:].rearrange("(c p) s -> p c s", p=128)

                def load_xt(sc, qs, eng=None):
                    t = XT.tile([128, 4, 512], MMD, tag="xt", name=f"xt_{sc}")
                    (eng or nc.sync).dma_start(out=t, in_=xt_r[:, :, qs])
                    return [t[:, c, :] for c in range(4)]

                def rope_proj_q(xt_c, qs, prologue=False):
                    # one 512-col chunk of rotated q^T. Prologue: dual
                    # projection (parallel chains, shortest latency).
                    # Steady state: permutation-matmul variant (5 PE
                    # matmuls instead of 8; qtmp evac on DVE).
                    ps1 = PPS.tile([128, 512], F32, tag="p1", name=f"p1_q{qs}")
                    for c in range(4):
                        nc.tensor.matmul(
                            ps1,
                            w_sb["wq"][:, c, :],
                            xt_c[c][:],
                            start=(c == 0),
                            stop=(c == 3),
                        )
                    if qk_bias:
                        s1 = W.tile([128, 512], F32, tag="rope1")
                        nc.vector.tensor_scalar_add(s1, ps1, bias_sb["qb"])
                    else:
                        s1 = ps1
                    if not prologue and use_perm:
                        qtmp = W.tile([128, 512], MMD, tag="ktmp")
                        nc.vector.tensor_copy(out=qtmp, in_=s1)
                    t3 = W.tile([128, 512], F32, tag="rope3")
                    nc.vector.tensor_tensor(out=t3, in0=s1, in1=cos_sb[:, qs], op=MUL)
                    if prologue:
                        ps2 = PSS.tile([128, 512], F32, tag="s", name=f"p2_q{qs}")
                    else:
                        ps2 = PPS.tile([128, 512], F32, tag="p1", name=f"p2_q{qs}")
                    if not prologue and use_perm:
                        nc.tensor.matmul(ps2, perm_sb[:, :], qtmp, start=True, stop=True)
                    else:
                        for c in range(4):
                            nc.tensor.matmul(
                                ps2,
                                w_sb["wqp"][:, c, :],
                                xt_c[c][:],
                                start=(c == 0),
                                stop=(c == 3),
                            )
                    if qk_bias:
                        s2 = W.tile([128, 512], F32, tag="rope2")
                        nc.vector.tensor_scalar_add(s2, ps2, bias_sb["qbp"])
                    else:
                        s2 = ps2
                    t4 = W.tile([128, 512], F32, tag="rope4")
                    nc.vector.tensor_tensor(out=t4, in0=s2, in1=sin_sb[:, qs], op=MUL)
                    nc.vector.tensor_tensor(out=qr[:, qs], in0=t3, in1=t4, op=ADD)

                def rope_proj_k_dual(xt_c, qs):
                    ps1 = PPS.tile([128, 512], F32, tag="p1", name=f"p1_k{qs}")
                    for c in range(4):
                        nc.tensor.matmul(
                            ps1,
                            w_sb["wk"][:, c, :],
                            xt_c[c][:],
                            start=(c == 0),
                            stop=(c == 3),
                        )
                    if qk_bias:
                        s1 = W.tile([128, 512], F32, tag="rope1")
                        nc.vector.tensor_scalar_add(s1, ps1, bias_sb["kb"])
                    else:
                        s1 = ps1
                    t3 = W.tile([128, 512], F32, tag="rope3")
                    nc.vector.tensor_tensor(out=t3, in0=s1, in1=cos_sb[:, qs], op=MUL)
                    ps2 = PPS.tile([128, 512], F32, tag="p1", name=f"p2_k{qs}")
                    for c in range(4):
                        nc.tensor.matmul(
                            ps2,
                            w_sb["wkp"][:, c, :],
                            xt_c[c][:],
                            start=(c == 0),
                            stop=(c == 3),
                        )
                    if qk_bias:
                        s2 = W.tile([128, 512], F32, tag="rope2")
                        nc.vector.tensor_scalar_add(s2, ps2, bias_sb["kbp"])
                    else:
                        s2 = ps2
                    t4 = W.tile([128, 512], F32, tag="rope4")
                    nc.vector.tensor_tensor(out=t4, in0=s2, in1=sin_sb[:, qs], op=MUL)
                    nc.gpsimd.tensor_tensor(
                        out=krA[0:HD, qs], in0=t3[0:HD, :], in1=t4[0:HD, :], op=ADD
                    )
                    nc.gpsimd.tensor_tensor(
                        out=krB[HD:DPC, qs],
                        in0=t3[HD:DPC, :],
                        in1=t4[HD:DPC, :],
                        op=ADD,
                    )

                def v_proj(xt_c, sc):
                    if v_bias:
                        for stl in range(4):
                            st = sc * 4 + stl
                            psv = PSS.tile([128, 128], F32, tag="s", name=f"pv{st}")
                            for c in range(4):
                                nc.tensor.matmul(
                                    psv,
                                    xt_c[c][:, bass.ts(stl, 128)],
                                    w_sb["wv"][:, c, :],
                                    start=(c == 0),
                                    stop=(c == 3),
                                )
                            dsts = vb_sb[:, st, :].rearrange(
                                "p (j w) -> p j w", w=HD + 1
                            )[:, :, 0:HD]
                            nc.vector.tensor_tensor(
                                out=dsts, in0=psv, in1=vbias_bc, op=ADD
                            )
                        return
                    pv4 = PSS.tile([128, 4, 128], F32, tag="s", name=f"pv4_{sc}")
                    for stl in range(4):
                        for c in range(4):
                            nc.tensor.matmul(
                                pv4[:, stl, :],
                                xt_c[c][:, bass.ts(stl, 128)],
                                w_sb["wv"][:, c, :],
                                start=(c == 0),
                                stop=(c == 3),
                            )
                    dsts = vb_sb[:, bass.ds(sc * 4, 4), :].rearrange(
                        "p s (j w) -> p s j w", w=HD + 1
                    )[:, :, :, 0:HD]
                    src = pv4[:].rearrange("p s (j w) -> p s j w", w=HD)
                    nc.scalar.copy(out=dsts, in_=src)

                # DMA issue order = first-need order: q weights -> xt0 ->
                # cos/sin chunk 0 -> k/v weights -> perm/ident/wo.
                qs0 = bass.ts(0, 512)
                with tc.high_priority():
                    xt_c0 = load_xt(0, qs0)
                    load_trig(qs0)
                load_w("wk")
                if use_perm:
                    # permm before wv: it gates the first k-rope chain
                    perm_sb = WP.tile([DPC, DPC], MMD, tag="permm")
                    nc.sync.dma_start(out=perm_sb, in_=perm_e[:])
                else:
                    load_w("wkp")
                load_w("wv")
                ident_sb = WP.tile([128, 128], MMD, tag="ident")
                nc.sync.dma_start(out=ident_sb, in_=ident_e[:])
                wo_sb = WP.tile([DPC, DIM], MMD, tag="wo")
                nc.sync.dma_start(out=wo_sb, in_=wo_e[:])

                # PE warmup during the initial DMA wait (clock-gate ramp)
                ps_w = PPS.tile([128, 512], F32, tag="p1", name="warm")
                wq_t = w_sb["wq"]
                for wi in range(8):
                    nc.tensor.matmul(
                        ps_w[:, 0:128],
                        wq_t[:, 0, 0:128],
                        wq_t[:, wi % 4, :],
                        start=True,
                        stop=True,
                    )
                # ACT table warmup: load the Exp set during the DMA wait so
                # the first real exp doesn't pay the ~2.7us table load (the
                # ktmp Copy below is in every table set, so no reload).
                warm_ex = W.tile([128, 1], MMD, tag="wex", bufs=1)
                nc.scalar.activation(
                    out=warm_ex, in_=ps_w[:, 0:1], func=EXP, scale=0.0001
                )

                rope_proj_q(xt_c0, qs0, prologue=True)
                xt_chunks = []
                for sc in range(NSC):
                    qs = bass.ts(sc, 512)
                    xt_c = load_xt(8 + sc, qs)
                    xt_chunks.append(xt_c)
                    if sc + 1 < NSC:
                        nqs = bass.ts(sc + 1, 512)
                        load_trig(nqs)
                    if use_perm:
                        # K rope via permutation matmul: k2 = P @ k1 with k1
                        # evacuated to SBUF by the (idle in this phase) ACT
                        ps1 = PPS.tile([128, 512], F32, tag="p1", name=f"p1_k{sc}")
                        for c in range(4):
                            nc.tensor.matmul(
                                ps1,
                                w_sb["wk"][:, c, :],
                                xt_c[c][:],
                                start=(c == 0),
                                stop=(c == 3),
                            )
                        ktmp = W.tile([128, 512], MMD, tag="ktmp")
                        nc.scalar.copy(out=ktmp, in_=ps1)
                        t3 = W.tile([128, 512], F32, tag="rope3")
                        nc.vector.tensor_tensor(
                            out=t3, in0=ps1, in1=cos_sb[:, qs], op=MUL
                        )
                        ps2 = POU.tile([128, 512], F32, tag="o", name=f"p2_k{sc}")
                        nc.tensor.matmul(
                            ps2, perm_sb[:, :], ktmp, start=True, stop=True
                        )
                        t4 = W.tile([128, 512], F32, tag="rope4")
                        nc.vector.tensor_tensor(
                            out=t4, in0=ps2, in1=sin_sb[:, qs], op=MUL
                        )
                        nc.vector.tensor_tensor(
                            out=krA[0:HD, qs], in0=t3[0:HD, :], in1=t4[0:HD, :], op=ADD
                        )
                        nc.vector.tensor_tensor(
                            out=krB[HD:DPC, qs],
                            in0=t3[HD:DPC, :],
                            in1=t4[HD:DPC, :],
                            op=ADD,
                        )
                    else:
                        rope_proj_k_dual(xt_c, qs)
                # V projections deferred behind all K chunks: PV consumes
                # vb late (filler), while the exp stream unthrottles as
                # soon as kr is complete.
                with tc.high_priority(offset=-(10**6)):
                    for sc in range(NSC):
                        v_proj(xt_chunks[sc], sc)

                # ---- attention: scores^T -> exp (ACT/DVE split) ->
                # transposed PV (O[q,d] + Z col) -> 1/Z fold into evac ->
                # PE transpose back -> single out-proj matmul per tile ----
                def emit_tile(qt, h, kc2, hoist):
                    qs = bass.ts(qt, 512)
                    krp = krA if h == 0 else krB
                    pss_t = PSS.tile(
                        [128, 1024], F32, tag="s", name=f"s{qt}_{h}_{kc2}"
                    )
                    sctx = tc.high_priority(offset=2000) if hoist else None
                    if sctx is not None:
                        sctx.__enter__()
                    for j in range(2):
                        kc = kc2 * 2 + j
                        nc.tensor.matmul(
                            pss_t[:, bass.ts(j, 512)],
                            krp[:, bass.ts(kc, 128)],
                            qr[:, qs],
                            start=True,
                            stop=True,
                        )
                    if sctx is not None:
                        sctx.__exit__(None, None, None)
                    pt = W.tile(
                        [128, 1024], MMD, tag="pt", bufs=32,
                        name=f"pt{qt}_{h}_{kc2}",
                    )
                    if kc2 in DVE_KC2:
                        # Schraudolph exp on DVE: bf16 bits of
                        # exp(s_raw/8) ~= int16(s'' + 16256)
                        nc.vector.tensor_scalar_add(pt[:].bitcast(I16), pss_t, B16)
                    else:
                        nc.scalar.activation(
                            out=pt, in_=pss_t, func=EXP, scale=ACT_SCALE
                        )
                    return pt

                def emit_pv_tail(qt, h, pts, ott, pso):
                    vcol = slice(h * (HD + 1), (h + 1) * (HD + 1))
                    # transposed PV: j4-OUTER so the 4 accumulation groups
                    # in the pso bank are sequential, never interleaved (a
                    # start re-marks the whole 2KB zero region pending).
                    # Filler priority: never delay the score stream.
                    with tc.high_priority(offset=-(10**6)):
                        for j4 in range(4):
                            for kc in range(NKC):
                                jh = kc % 2
                                nc.tensor.matmul(
                                    pso[:, j4, :],
                                    pts[kc // 2][
                                        :,
                                        jh * 512 + j4 * 128 : jh * 512 + (j4 + 1) * 128,
                                    ],
                                    vb_sb[:, kc, vcol],
                                    start=(kc == 0),
                                    stop=(kc == NKC - 1),
                                )
                    # 1/Z (Z = col 64 of each subtile, on partitions)
                    with tc.high_priority(offset=-(10**6)):
                        izt = W.tile([128, 4], F32, tag="izt", bufs=4)
                        nc.vector.reciprocal(out=izt, in_=pso[:, :, HD : HD + 1])
                        # normalized evac O*1/Z -> bf16 [128, 4, 64]
                        osb = W.tile([128, 4, HD], MMD, tag="osb", bufs=4)
                        izb = bass.AP(
                            tensor=izt.tensor,
                            offset=izt.offset,
                            ap=[izt.ap[0], [1, 4], [0, HD]],
                        )
                        nc.vector.tensor_tensor(
                            out=osb, in0=pso[:, :, 0:HD], in1=izb, op=MUL
                        )
                        # PE transpose back: [128 q, 64 d] -> [64 d, 128 q]
                        # head h lands on partitions h*64:(h+1)*64 so the
                        # DVE evac keeps in/out partition offsets aligned
                        otp = PPS.tile(
                            [128, 4, 128], MMD, tag="p1", name=f"tp{qt}_{h}"
                        )
                        hs = slice(h * HD, (h + 1) * HD)
                        for j4 in range(4):
                            nc.tensor.matmul(
                                otp[hs, j4, :],
                                osb[:, j4, :],
                                ident_sb,
                                is_transpose=True,
                                start=True,
                                stop=True,
                            )
                        nc.vector.tensor_copy(
                            out=ott[hs, :],
                            in_=otp[hs, :, :].rearrange("p a b -> p (a b)"),
                        )

                for qt in range(NSC):
                    last = qt == NSC - 1
                    if not last:
                        with tc.high_priority(offset=-(2 * 10**5)):
                            # xt chunk qt+1 is still resident from the
                            # projection phase (XT ring holds all 9)
                            qs_n = bass.ts(qt + 1, 512)
                            rope_proj_q(xt_chunks[qt + 1], qs_n)
                    ott = W.tile([128, 512], MMD, tag="ott", bufs=3, name=f"ott{qt}")
                    for h in range(HPC):
                        pso = POU.tile(
                            [128, 4, HD + 1], F32, tag="o", bufs=1, name=f"o{qt}_{h}"
                        )
                        pts = [
                            emit_tile(qt, h, kc2, False) for kc2 in range(KC2)
                        ]
                        emit_pv_tail(qt, h, pts, ott, pso)
                    # output projection: both heads in one matmul per tile
                    stk = tc.high_priority(offset=-(10**6))
                    stk.__enter__()
                    for utl in range(4):
                        ut = qt * 4 + utl
                        us = bass.ts(ut, 128)
                        if last:
                            # PSS ring is idle at the drain: parallel chains
                            psu = PSS.tile([128, DIM], F32, tag="s", name=f"u_{ut}")
                        else:
                            psu = PPS.tile([128, DIM], F32, tag="p1", name=f"u_{ut}")
                        nc.tensor.matmul(
                            psu, ott[:, bass.ts(utl, 128)], wo_sb, start=True, stop=True
                        )
                        u_sb = W.tile([128, DIM], F32, tag="uout")
                        if last:
                            # ACT is idle at the drain; Copy shares the Exp
                            # table set (no reload)
                            nc.scalar.copy(out=u_sb, in_=psu)
                        else:
                            nc.vector.tensor_copy(out=u_sb, in_=psu)
                        nc.sync.dma_start(out=out_e[us, :], in_=u_sb)
                    stk.__exit__(None, None, None)

    return nc


def _rope_tables():
    freqs = 10000.0 ** (-np.linspace(0.0, 1.0, HALF, endpoint=False))
    theta = np.arange(S, dtype=np.float64)[None, :] * freqs[:, None]  # [32, S]
    cos32 = np.cos(theta)
    sin32 = np.sin(theta)
    return cos32, np.concatenate([-sin32, sin32], axis=0)


def kernel(x, wq_k, wq_b, wk_k, wk_b, wv_k, wv_b, wo_k, wo_b):
    from concourse.bass_utils import run_bass_kernel_spmd

    x = np.asarray(x, np.float32)
    wq_k = np.asarray(wq_k, np.float32) * np.float32(QSCALE)
    wq_b = np.asarray(wq_b, np.float32) * np.float32(QSCALE)
    wk_k = np.asarray(wk_k, np.float32)
    wk_b = np.asarray(wk_b, np.float32)
    wv_k = np.asarray(wv_k, np.float32)
    wv_b = np.asarray(wv_b, np.float32)
    wo_k = np.asarray(wo_k, np.float32)
    wo_b = np.asarray(wo_b, np.float32)

    qk_bias = bool(np.any(wq_b) or np.any(wk_b))
    v_bias = bool(np.any(wv_b))

    key = (qk_bias, v_bias)
    if key not in _CACHE:
        nc = _build(qk_bias, v_bias)
        _split_multiwait_drains(nc)
        _CACHE[key] = nc
    nc = _CACHE[key]
    import ml_dtypes

    mmdt = ml_dtypes.bfloat16

    cosf, sinf = _rope_tables()
    cosf = np.ascontiguousarray(cosf).astype(mmdt)
    sinf = np.ascontiguousarray(sinf).astype(mmdt)
    perm = np.r_[HALF:HD, 0:HALF]
    use_perm = not qk_bias
    if use_perm:
        permm = np.zeros((DPC, DPC), np.float32)
        for d in range(DPC):
            permm[(d % HD + HALF) % HD + HD * (d // HD), d] = 1.0
        permm = permm.astype(mmdt)
    ident = np.eye(128, dtype=np.float32).astype(mmdt)

    in_maps = []
    for c in range(NCORES):
        b = c // 4
        h0 = HPC * (c % 4)
        hsl = slice(h0, h0 + HPC)
        m = {
            "xt": np.ascontiguousarray(x[b].T).astype(mmdt),
            "wq": np.ascontiguousarray(wq_k[:, hsl, :].reshape(DIM, DPC)).astype(mmdt),
            "wqp": np.ascontiguousarray(wq_k[:, hsl, perm].reshape(DIM, DPC)).astype(mmdt),
            "wk": np.ascontiguousarray(wk_k[:, hsl, :].reshape(DIM, DPC)).astype(mmdt),
            "wv": np.ascontiguousarray(wv_k[:, hsl, :].reshape(DIM, DPC)).astype(mmdt),
            "wo": np.ascontiguousarray(wo_k[hsl].reshape(DPC, DIM)).astype(mmdt),
            "cosf": cosf,
            "sinf": sinf,
            "ident": ident,
        }
        if use_perm:
            m["permm"] = permm
        else:
            m["wkp"] = np.ascontiguousarray(
                wk_k[:, hsl, perm].reshape(DIM, DPC)
            ).astype(mmdt)
        if qk_bias:
            m["qb"] = np.ascontiguousarray(wq_b[hsl].reshape(DPC, 1))
            m["qbp"] = np.ascontiguousarray(wq_b[hsl][:, perm].reshape(DPC, 1))
            m["kb"] = np.ascontiguousarray(wk_b[hsl].reshape(DPC, 1))
            m["kbp"] = np.ascontiguousarray(wk_b[hsl][:, perm].reshape(DPC, 1))
        if v_bias:
            m["vb"] = np.ascontiguousarray(wv_b[hsl].reshape(1, DPC))
        in_maps.append(m)

    res = run_bass_kernel_spmd(nc, in_maps, list(range(NCORES)))
    globals()["_LAST_RESULTS"] = res

    out = np.zeros((B, S, DIM), np.float32)
    for c in range(NCORES):
        out[c // 4] += res.results[c]["out"].astype(np.float32)
    out += wo_b[None, None, :]
    return out
